# revision 50
# baseline (speedup 1.0000x reference)
"""Trainium2 Bass kernel for DrafterAttention (decode attention, B=8 H=16 D=128 S=4096 HID=2048).

Strategy (tensor-parallel over heads, 8 cores x 2 heads):
  - Host: shard Wq columns / Wo rows / kv on the head axis; pre-transpose
    kv_k -> (B,HC,D,S) and pre-tile kv_v -> (B,HC,128,NCH*128); quantize both
    to fp8-e3m4 (1 byte/elt halves HBM traffic vs bf16; 4 mantissa bits keep
    the output inside the 2e-2 gate). Wq/Wo/x are pre-packed on host into the
    exact SBUF layout so every device DMA is one contiguous 128-partition
    transfer.
  - Device (per core): the whole kv shard (16 units x 8KB/partition) fits in
    SBUF, so all kv DMAs are issued up front on the sync ring (unit-pair
    transfers, 8KB partition lines) and compute is fully decoupled from the
    stream. qT = Wq_shard^T @ x^T on the PE; RMS-norm + RoPE in a
    (d-on-partition, batch-on-free) layout; per (b,h): 32 matmuls
    kT_chunk^T @ q_col -> scores (128s x 32chunk) in one PSUM accumulation
    group with the mask add folded in via an f16 identity weight (exp reads
    PSUM directly; no max subtraction: logits are O(1) by construction);
    AV runs 2 units behind scores so the ACT/DVE softmax chain is hidden
    even in the post-stream drain phase; softmax normalization is a
    one-directional PE->DVE flow (ones-matmul colsum broadcast, DVE
    free-axis reduce + reciprocal + scale); o_proj is computed transposed
    (chunk-of-128-columns on partitions) so DVE ops use all 128 lanes, and
    the host un-transposes during the all-reduce.
  - Host: sum the 8 partial outputs (the all-reduce) and un-transpose.

K/V are fp8-e3m4 matmul weights; q and probs are fp16 moving operands
(mixed-dtype matmuls are legal when neither side is fp32). Accumulation is
always fp32 in PSUM; softmax statistics are fp32.
"""
import numpy as np

B, H, D, S, HID = 8, 16, 128, 4096, 2048
NCORES = 8
HC = H // NCORES          # 2 heads per core
NCH = S // 128            # 32 s-chunks
SCALE = 1.0 / np.sqrt(D)
EPS = 1e-6

K_DTYPE = "f8e3"          # "f8e3" | "bf16"
V_DTYPE = "f8e3"          # "f8e3" | "bf16"
KV_BUFS = 16              # units resident in SBUF (16 = whole shard)

_CACHE = {}


def _split_sync_waits(nc, max_waits=1):
    """This walrus build caps per-instruction sem waits; hoist any excess
    onto NoOp instructions inserted just before, on the same engine."""
    from concourse import mybir
    import bass_rust

    n = 0
    for fn in nc.m.functions:
        for blk in fn.blocks:
            new_list = []
            changed = False
            for inst in blk.instructions:
                si = inst.sync_info
                waits = list(si.on_wait) if (si and si.on_wait) else []
                if len(waits) > max_waits:
                    extra, keep = waits[:-max_waits], waits[-max_waits:]
                    for i in range(0, len(extra), max_waits):
                        n += 1
                        nop = bass_rust.InstNoOp(
                            name=f"I-waitsplit-{n}", ins=[], outs=[])
                        nop.engine = inst.engine
                        nop.sync_info = mybir.SyncInfo(
                            on_wait=extra[i:i + max_waits], on_update=[])
                        new_list.append(nop)
                    si.on_wait = keep
                    changed = True
                new_list.append(inst)
            if changed:
                blk.instructions[:] = new_list
    return n


def _mybir_dt(name):
    from concourse import mybir
    return {"f8e3": mybir.dt.float8e3, "bf16": mybir.dt.bfloat16}[name]


def _build_nc(k_dtype=K_DTYPE, v_dtype=V_DTYPE):
    from contextlib import ExitStack
    import concourse.bass as bass
    import concourse.tile as tile
    from concourse import mybir

    f32 = mybir.dt.float32
    f16 = mybir.dt.float16
    bf16 = mybir.dt.bfloat16
    k_dt = _mybir_dt(k_dtype)
    v_dt = _mybir_dt(v_dtype)

    nc = bass.Bass(trn_type="TRN2")

    # kv packed per unit pair (8KB partition lines amortize the per-packet
    # SDMA overhead): kp[j,p,i*S+s] = K[unit 2j+i][s,p] with unit u=(h,b)
    # h-major; vp[j,p,i*S+c*128+d] = V[unit 2j+i][d,c*128+p]
    NU = B * HC
    kp = nc.dram_tensor("kp", [NU // 2, 128, 2 * S], k_dt, kind="ExternalInput")
    vp = nc.dram_tensor("vp", [NU // 2, 128, 2 * S], v_dt, kind="ExternalInput")
    # host-packed to SBUF layout: wqp[p, i*HCD + j] = Wq[i*128+p, hs*D + j]
    wqp = nc.dram_tensor("wqp", [128, (HID // 128) * HC * D], bf16,
                         kind="ExternalInput")
    # wop[p, h*HID + n] = Wo[(hs+h)*D + p, n]
    wop = nc.dram_tensor("wop", [128, HC * HID], bf16, kind="ExternalInput")
    # xTp[p, i*B+b] = x[b, i*128+p]
    xtp = nc.dram_tensor("xtp", [128, (HID // 128) * B], f16,
                         kind="ExternalInput")
    # smalls: col0 = [cos;sin], col1 = gamma*SCALE
    sm = nc.dram_tensor("sm", [128, 2], f32, kind="ExternalInput")
    # mask tiles (f16) and f16 identity: the mask add is folded into the
    # score matmul group (identity weight) so exp reads PSUM directly
    maskp = nc.dram_tensor("maskp", [128, B * NCH], f16, kind="ExternalInput")
    idp = nc.dram_tensor("idp", [128, 128], f16, kind="ExternalInput")
    # transposed output: out[p, c*B + b] = result[b, c*128 + p]
    out = nc.dram_tensor("out", [128, (HID // 128) * B], f32,
                         kind="ExternalOutput")

    units = [(h, b) for h in range(HC) for b in range(B)]

    with ExitStack() as ctx:
        tc = ctx.enter_context(tile.TileContext(nc))

        consts = ctx.enter_context(tc.tile_pool(name="consts", bufs=1))
        kpool = ctx.enter_context(tc.tile_pool(name="kpool", bufs=KV_BUFS // 2))
        vpool = ctx.enter_context(tc.tile_pool(name="vpool", bufs=KV_BUFS // 2))
        prpool = ctx.enter_context(tc.tile_pool(name="prpool", bufs=6))
        stpool = ctx.enter_context(tc.tile_pool(name="stpool", bufs=6))

        # ---- kv stream: the whole shard is prefetched to SBUF on the sync
        # ring (it is otherwise idle); one DMA per unit PAIR per k/v half ----
        # Issue order: interleave k/v pairs for the bulk, but pull the last
        # two k pairs ahead of the last two v transfers and land the final
        # two v units as singles — after the last byte only the final
        # unit's AV chain remains (scores/exps complete during the last v
        # transfers).
        NP = len(units) // 2
        k_pairs = [kpool.tile([128, 2 * S], k_dt, name="ktile")
                   for _ in range(NP)]
        v_pairs = [vpool.tile([128, 2 * S], v_dt, name="vtile")
                   for _ in range(NP)]
        for j in range(NP - 2):
            nc.sync.dma_start(k_pairs[j][:], kp[j])
            nc.sync.dma_start(v_pairs[j][:], vp[j])
        nc.sync.dma_start(k_pairs[NP - 2][:], kp[NP - 2])
        nc.sync.dma_start(k_pairs[NP - 1][:], kp[NP - 1])
        nc.sync.dma_start(v_pairs[NP - 2][:], vp[NP - 2])
        nc.sync.dma_start(v_pairs[NP - 1][:, 0:S], vp[NP - 1][:, 0:S])
        nc.sync.dma_start(v_pairs[NP - 1][:, S:2 * S], vp[NP - 1][:, S:2 * S])
        # per-unit (pair_tile, column_base) accessors
        k_tiles = [(k_pairs[u // 2], (u % 2) * S) for u in range(len(units))]
        v_tiles = [(v_pairs[u // 2], (u % 2) * S) for u in range(len(units))]

        # ---- weights / smalls on the scalar ring ----
        wq_sb = consts.tile([128, HID // 128, HC * D], bf16)
        nc.scalar.dma_start(
            wq_sb[:], wqp[:].rearrange("p (i j) -> p i j", j=HC * D))
        xT_sb = consts.tile([128, HID // 128, B], f16)
        nc.scalar.dma_start(
            xT_sb[:], xtp[:].rearrange("p (i b) -> p i b", b=B))
        sm_sb = consts.tile([128, 2], f32)
        nc.scalar.dma_start(sm_sb[:], sm[:])
        mask_sb = consts.tile([128, B * NCH], f16)
        nc.scalar.dma_start(mask_sb[:], maskp[:])
        id_sb = consts.tile([128, 128], f16)
        nc.scalar.dma_start(id_sb[:], idp[:])
        wo_sb = consts.tile([128, HC, HID], bf16)
        nc.scalar.dma_start(
            wo_sb[:], wop[:].rearrange("p (h n) -> p h n", n=HID))

        ones_sb = consts.tile([128, 128], f32)
        nc.gpsimd.memset(ones_sb[:], 1.0)
        onesb_sb = consts.tile([128, 128], bf16)
        nc.gpsimd.memset(onesb_sb[:], 1.0)
        eps_sb = consts.tile([128, 1], f32)
        nc.gpsimd.memset(eps_sb[:], EPS)
        mask_tiles = [mask_sb[:, b * NCH:(b + 1) * NCH] for b in range(B)]

        # ---- q projection: qT_h = (Wq_h)^T @ x^T  -> (128d, B) per head ----
        qpool = ctx.enter_context(tc.tile_pool(name="qpool", bufs=1))
        q_heads = []
        with tc.tile_pool(name="psQ", bufs=1, space="PSUM") as psq:
            for h in range(HC):
                q_ps = psq.tile([128, B], f32, name="qps")
                for i in range(HID // 128):
                    nc.tensor.matmul(
                        q_ps[:],
                        wq_sb[:, i, h * D:(h + 1) * D],
                        xT_sb[:, i, :],
                        start=(i == 0), stop=(i == HID // 128 - 1),
                    )
                # RMS norm (over the partition axis d) via ones-matmul
                qs = qpool.tile([128, 3 * B], f32, name=f"qs{h}")
                sq = qs[:, 0:B]
                rms = qs[:, B:2 * B]
                qn = qs[:, 2 * B:3 * B]
                nc.scalar.square(sq, q_ps[:])
                ssq_ps = psq.tile([128, B], f32, name="ssq")
                nc.tensor.matmul(ssq_ps[:], ones_sb[:], sq, start=True, stop=True)
                nc.scalar.activation(rms, ssq_ps[:],
                                     mybir.ActivationFunctionType.Sqrt,
                                     bias=eps_sb[:], scale=1.0 / D)
                nc.vector.reciprocal(rms, rms)
                nc.vector.tensor_mul(qn, q_ps[:], rms)
                # gamma * SCALE (per-partition scalar)
                nc.vector.tensor_scalar_mul(qn, qn, sm_sb[:, 1:2])
                # RoPE on partition halves: cos/sin stacked in sm col 0;
                # t1/t2 reuse the dead sq/rms columns
                qr = qpool.tile([128, B], f16, name=f"qr{h}")
                t1 = qs[0:64, 0:B]
                t2 = qs[0:64, B:2 * B]
                cos_ap = sm_sb[0:64, 0:1]
                sin_ap = sm_sb[64:128, 0:1]
                q1 = qn[0:64, :]
                q2 = qn[64:128, :]
                nc.vector.tensor_scalar_mul(t1, q1, cos_ap)
                nc.vector.tensor_scalar_mul(t2, q2, sin_ap)
                nc.vector.tensor_sub(qr[0:64, :], t1, t2)
                nc.vector.tensor_scalar_mul(t1, q2, cos_ap)
                nc.vector.tensor_scalar_mul(t2, q1, sin_ap)
                nc.vector.tensor_add(qr[64:128, :], t1, t2)
                q_heads.append(qr)

        # attention output columns, (128d, B) per head
        at_tiles = [qpool.tile([128, B], f16, name=f"at{h}") for h in range(HC)]

        ps_sc = ctx.enter_context(tc.tile_pool(name="psS", bufs=3, space="PSUM"))
        ps_av = ctx.enter_context(tc.tile_pool(name="psV", bufs=3, space="PSUM"))
        ps_o = ctx.enter_context(tc.tile_pool(name="psO", bufs=1, space="PSUM"))
        # o_proj in transposed layout: per chunk c, out[n, b] over the 128
        # n-columns of the chunk — uses all 128 partitions/DVE lanes.
        oT_sb = qpool.tile([128, (HID // 128) * B], f32, name="oT")

        def emit_oproj(h):
            o_ps = ps_o.tile([128, (HID // 128) * B], f32, name="ops")
            for c in range(HID // 128):
                nc.tensor.matmul(
                    o_ps[:, c * B:(c + 1) * B],
                    wo_sb[:, h, c * 128:(c + 1) * 128],
                    at_tiles[h][:],
                    start=True, stop=True,
                )
            if h == 0:
                nc.vector.tensor_copy(oT_sb[:], o_ps[:])
            else:
                nc.vector.tensor_add(oT_sb[:], oT_sb[:], o_ps[:])

        def emit_av(pend, last=False):
            # AV + softmax normalization, one-directional PE -> DVE flow:
            # colsum matmul broadcasts per-chunk prob sums to all partitions,
            # DVE reduces/reciprocates/scales. No ACT involvement. For the
            # final unit the colsum goes first so the DVE reduce/reciprocal
            # overlap the AV burst on the critical tail.
            (v_p, vo), probs_p, stats_p, h_p, b_p = pend
            avcs = ps_av.tile([128, 1 + NCH], f32, name="avps")
            av_ps = avcs[:, 0:1]
            cs_ps = avcs[:, 1:1 + NCH]

            def cs_mm():
                nc.tensor.matmul(cs_ps, onesb_sb[:], probs_p[:],
                                 start=True, stop=True)

            tot = stats_p[:, 0:1]
            inv = stats_p[:, 1:2]
            if last:
                cs_mm()
                nc.vector.tensor_reduce(tot, cs_ps, mybir.AxisListType.X,
                                        mybir.AluOpType.add)
                nc.vector.reciprocal(inv, tot)
            for c in range(NCH):
                nc.tensor.matmul(
                    av_ps,
                    v_p[:, vo + c * 128:vo + (c + 1) * 128],
                    probs_p[:, c:c + 1],
                    start=(c == 0), stop=(c == NCH - 1),
                )
            if not last:
                cs_mm()
                nc.vector.tensor_reduce(tot, cs_ps, mybir.AxisListType.X,
                                        mybir.AluOpType.add)
                nc.vector.reciprocal(inv, tot)
            nc.vector.tensor_scalar_mul(at_tiles[h_p][:, b_p:b_p + 1],
                                        av_ps[:], inv)

        # ---- main attention loop (h-major; AV pipelined 2 units behind
        # scores so the PE never waits on the ACT/DVE softmax chain, even
        # in the post-stream drain phase) ----
        pendings = []
        for u, (h, b) in enumerate(units):
            q_col = q_heads[h][:, b:b + 1]
            k_sb, ko = k_tiles[u]
            sc_ps = ps_sc.tile([128, NCH], f32, name="scps")
            for c in range(NCH):
                nc.tensor.matmul(
                    sc_ps[:, c:c + 1],
                    k_sb[:, ko + c * 128:ko + (c + 1) * 128],
                    q_col,
                    start=(c == 0), stop=False,
                )
            # mask add folded into the group: sc += I.T @ mask_tile
            nc.tensor.matmul(sc_ps[:], id_sb[:], mask_tiles[b],
                             start=False, stop=True)
            # Keep AV two units behind scores mid-stream; for the last two
            # units defer the AVs past the final scores so S(14)/S(15) and
            # their exps are not FIFO-blocked behind AVs waiting on the
            # late v transfers.
            if len(pendings) == 2 and u < len(units) - 2:
                emit_av(pendings.pop(0))
            stats = stpool.tile([128, 2], f32, name="stats")
            probs = prpool.tile([128, NCH], f16, name="probs")
            nc.scalar.activation(probs[:], sc_ps[:],
                                 mybir.ActivationFunctionType.Exp)
            pendings.append((v_tiles[u], probs, stats, h, b))
            if u == B + 3:
                # head 0's attention columns are complete; open the o_proj
                # accumulation groups mid-loop
                emit_oproj(0)
        for i, p in enumerate(pendings):
            emit_av(p, last=(i == len(pendings) - 1))
        emit_oproj(1)
        nc.scalar.dma_start(out[:], oT_sb[:])

    _split_sync_waits(nc)
    return nc


def _get_nc():
    if "nc" not in _CACHE:
        _CACHE["nc"] = _build_nc()
    return _CACHE["nc"]


def _np_dt(name):
    import ml_dtypes
    return {"f8e3": ml_dtypes.float8_e3m4, "bf16": ml_dtypes.bfloat16}[name]


def _shard_inputs(x, kv_k, kv_v, cos, sin, mask, Wq, Wo, q_gamma,
                  k_dtype=K_DTYPE, v_dtype=V_DTYPE):
    import ml_dtypes
    bf16 = ml_dtypes.bfloat16
    k_np = _np_dt(k_dtype)
    v_np = _np_dt(v_dtype)

    x = np.asarray(x, np.float32).reshape(B, HID)
    # xTp[p, i*B+b] = x[b, i*128+p]
    xtp = np.ascontiguousarray(
        x.reshape(B, HID // 128, 128).transpose(2, 1, 0)
        .reshape(128, (HID // 128) * B).astype(np.float16))
    sm = np.empty((128, 2), np.float32)
    sm[:64, 0] = np.asarray(cos, np.float32).reshape(-1)
    sm[64:, 0] = np.asarray(sin, np.float32).reshape(-1)
    sm[:, 1] = np.asarray(q_gamma, np.float32).reshape(-1) * SCALE
    maskp = np.ascontiguousarray(
        np.asarray(mask, np.float32).reshape(B, NCH, 128)
        .transpose(2, 0, 1).reshape(128, B * NCH).astype(np.float16))
    idp = np.eye(128, dtype=np.float16)
    # quantize once for the full tensors, then slice per core
    kq = np.asarray(kv_k, np.float32).astype(k_np)      # (B, H, S, D)
    vq = np.asarray(kv_v, np.float32).astype(v_np)      # (B, H, D, S)
    Wq = np.asarray(Wq, np.float32)
    Wo = np.asarray(Wo, np.float32)

    NU = B * HC
    in_maps = []
    for c in range(NCORES):
        hs = c * HC
        # per-unit (h-major) transposed slabs, paired along the line axis
        kt = (kq[:, hs:hs + HC].transpose(1, 0, 3, 2)     # (HC,B,D=p,S)
              .reshape(NU, 128, S))
        vt = (vq[:, hs:hs + HC].reshape(B, HC, D, NCH, 128)
              .transpose(1, 0, 4, 3, 2).reshape(NU, 128, S))
        kp = np.ascontiguousarray(
            kt.reshape(NU // 2, 2, 128, S).transpose(0, 2, 1, 3)
            .reshape(NU // 2, 128, 2 * S))
        vp = np.ascontiguousarray(
            vt.reshape(NU // 2, 2, 128, S).transpose(0, 2, 1, 3)
            .reshape(NU // 2, 128, 2 * S))
        # wqp[p, i*HCD + j] = Wq[i*128+p, hs*D + j]
        wqp = np.ascontiguousarray(
            Wq.reshape(HID // 128, 128, HID)[:, :, hs * D:(hs + HC) * D]
            .transpose(1, 0, 2).reshape(128, (HID // 128) * HC * D)
            .astype(bf16))
        # wop[p, h*HID + n] = Wo[(hs+h)*D + p, n]
        wop = np.ascontiguousarray(
            Wo[hs * D:(hs + HC) * D].reshape(HC, 128, HID)
            .transpose(1, 0, 2).reshape(128, HC * HID).astype(bf16))
        in_maps.append({
            "kp": kp,
            "vp": vp,
            "wqp": wqp,
            "wop": wop,
            "xtp": xtp,
            "sm": sm,
            "maskp": maskp,
            "idp": idp,
        })
    return in_maps


def kernel(x, kv_k, kv_v, cos, sin, mask, Wq, Wo, q_gamma, _trace=False):
    from concourse.bass_utils import run_bass_kernel_spmd

    nc = _get_nc()
    in_maps = _shard_inputs(x, kv_k, kv_v, cos, sin, mask, Wq, Wo, q_gamma)
    res = run_bass_kernel_spmd(nc, in_maps, list(range(NCORES)), trace=_trace)
    acc = np.zeros((128, (HID // 128) * B), np.float64)
    for c in range(NCORES):
        acc += res.results[c]["out"].astype(np.float64)
    # outT[p, c*B + b] -> out[b, c*128 + p]
    out = np.ascontiguousarray(
        acc.reshape(128, HID // 128, B).transpose(2, 1, 0)
        .reshape(B, 1, HID).astype(np.float32))
    if _trace:
        return out, res
    return out
